# revision 15
# baseline (speedup 1.0000x reference)
"""MoE block (B=16,N=1024,C=768,E=8,H=192,D=4,K=2) on 8 NeuronCores.

Data-parallel over B (2 samples/core). Per sample:
  - noisy gating in split-f16 (hi+lo gate weights, f16 x) with tokens on
    partitions -> tiny matmuls; ews reduced on-chip; top-2 via max8.
    For K=2 the scaled-softmax gates are constants sigmoid(1)/1-sigmoid(1)
    (scaled = [1,0] always), so gates are folded into pre-scaled fc2 weight
    copies on the host.
  - one fp8 row-gather per selected expert (fc1+fc2+bias packed in DoubleRow
    interleave), fp8 DoubleRow fc1 and fc2 (0.5 cyc/row), exact-Gelu with
    per-partition bias (incl. a gelu(z)=1 row that feeds the fc2 bias through
    the matmul), residual added either fused in the DVE drain or via an f16
    identity matmul into the fc2 PSUM group with the drain copy on Act,
    f16 output.

Host prep is pure value-preserving re-layout: transpose, dtype split
(f16 hi/lo, fp8), index-gather of gate_w by task_ids, weight packing with
the constant top-2 gate folded in.

Layouts shipped from host (per sample):
  xt16  [128, 6, 1024] f16   xT (c%128 on partitions, c//128 chunks)
  xt8   [128, 6, 1024] f8    same, fp8 (fc1 rhs; DoubleRow pairs = chunk pairs)
  gw    [128, 6, 32]   f16   gate weights: clean_hi|noise_hi|clean_lo|noise_lo
  epsd  [128, 64]      f32   eps[(t,p),e] -> [p, (t,e)]
  id128 [128, 128]     f16   identity (PE residual add)
  wpack [2*E*128, 3080] f8   per (gate-copy, expert) packed rows:
        [0:768)    fc1 m0 (kpair3, jj2, h0:128)
        [768:1536) fc1 m1 (kpair3, jj2, h128:192|zeros)
        [1536:3072) fc2 (cchunk6, jj2, c128)*gate (+bias row at p=64,jj=1)
        [3072]     fc1 bias h0:128   [3073] fc1 bias h128:192 | z*(gelu->1) | 0
"""
import numpy as np
import ml_dtypes

import concourse.bass as bass
import concourse.mybir as mybir
import concourse.tile as tile
from concourse import bacc
from concourse.bass_utils import run_bass_kernel_spmd

f16 = np.float16
f8 = ml_dtypes.float8_e4m3
f32 = np.float32
AF = mybir.ActivationFunctionType
ALU = mybir.AluOpType
PM = mybir.MatmulPerfMode
dt = mybir.dt

B, N, C = 16, 1024, 768
E, H, D, TOPK = 8, 192, 4, 2
NCORES = 8
SPC = B // NCORES          # samples per core = 2
C_K = C // 128             # 6 channel chunks
KP = C_K // 2              # 3 DoubleRow k-pairs
TCH = N // 128             # 8 token chunks
NT = N // 512              # 2 big n-chunks
PCK = 3080                 # packed row bytes (fp8)
G1 = float(1.0 / (1.0 + np.exp(-1.0)))
G2 = 1.0 - G1
ZSTAR = 1.125              # f8-exact; gelu(1.125)=0.978 -> f8 rounds to 1.0

# drain engine per (ci, n) chunk: DVE does fused residual adds; Act does
# pure copies (residual added by PE identity matmul into PSUM). GPSIMD has
# no PSUM port, so Pool cannot help here.
_DRAIN = ["dve", "act", "dve"] * 4

_cache = {}


def _build(reps=1):
    key = ("nc", reps)
    if key in _cache:
        return _cache[key]
    nc = bacc.Bacc("TRN2", target_bir_lowering=False, debug=False,
                   num_devices=NCORES)

    x16_d = nc.dram_tensor("xt16", [SPC, 128, C_K, N], dt.float16, kind="ExternalInput").ap()
    x8_d = nc.dram_tensor("xt8", [SPC, 128, C_K, N], dt.float8e4, kind="ExternalInput").ap()
    gw_d = nc.dram_tensor("gw", [SPC, 128, C_K, 32], dt.float16, kind="ExternalInput").ap()
    ep_d = nc.dram_tensor("epsd", [SPC, 128, TCH * E], dt.float32, kind="ExternalInput").ap()
    id_d = nc.dram_tensor("id128", [128, 128], dt.float16, kind="ExternalInput").ap()
    wp_d = nc.dram_tensor("wpack", [2 * E * 128, PCK], dt.float8e4, kind="ExternalInput").ap()
    y_d = nc.dram_tensor("y", [SPC, 128, C_K, N], dt.float16, kind="ExternalOutput").ap()

    with tile.TileContext(nc) as tc:
        with tc.tile_pool(name="const", bufs=1) as cp, \
             tc.tile_pool(name="xt", bufs=2) as xtp, \
             tc.tile_pool(name="gate", bufs=2) as gp, \
             tc.tile_pool(name="wt", bufs=2) as wtp, \
             tc.tile_pool(name="h", bufs=2) as hp, \
             tc.tile_pool(name="yout", bufs=2) as yp, \
             tc.tile_pool(name="ps_g", bufs=2, space="PSUM") as psg, \
             tc.tile_pool(name="ps_t", bufs=2, space="PSUM") as pst, \
             tc.tile_pool(name="ps_f1", bufs=2, space="PSUM") as psf, \
             tc.tile_pool(name="ps_y", bufs=2, space="PSUM") as psy:

            # constants (no DMA except id128)
            iota_i = cp.tile([128, 1], dt.int32, tag="iota_i")
            nc.gpsimd.iota(iota_i[:], pattern=[[0, 1]], base=0, channel_multiplier=1)
            iota_f = cp.tile([128, 1], dt.float32, tag="iota_f")
            nc.vector.tensor_copy(iota_f[:], iota_i[:])
            onesq = cp.tile([128, 128], dt.float32, tag="onesq")
            nc.vector.memset(onesq[:], 1.0)
            id128 = cp.tile([128, 128], dt.float16, tag="id128")
            # explicitly pre-load the exp+ln act table (set 6 =
            # natural_log_exp_and_others) so the softplus chain (Exp then
            # Ln, possibly interleaved by the scheduler) needs no implicit
            # table loads; the gelu table loads once later.
            nc.scalar.add_instruction(mybir.InstLoadActFuncSet(
                name=nc.get_next_instruction_name(), ins=[], outs=[],
                act_func_set_id=6))

            for rep in range(reps):
              # ---- loads: small tensors first, x chunks, fp8 x last ----
              gwts, xts = [], []
              eps_both = gp.tile([128, SPC, TCH * E], dt.float32, tag="epsb")
              for s in range(SPC):
                  gwt = gp.tile([128, C_K, 32], dt.float16, tag="gwt")
                  nc.sync.dma_start(gwt[:], gw_d[s])
                  nc.sync.dma_start(eps_both[:, s, :], ep_d[s])
                  gwts.append(gwt)
              for s in range(SPC):
                  xt16 = xtp.tile([128, C_K, N], dt.float16, tag="xt16",
                                  name="xt16")
                  for k in range(C_K):
                      nc.sync.dma_start(xt16[:, k, :], x16_d[s, :, k, :])
                  xts.append(xt16)
              x8s = []
              for s in range(SPC):
                  xt8 = xtp.tile([128, C_K, N], dt.float8e4, tag="xt8",
                                 name="xt8")
                  nc.sync.dma_start(xt8[:], x8_d[s])
                  x8s.append(xt8)
              if rep == 0:
                  nc.sync.dma_start(id128[:], id_d[:, :])

              # ---- gating, both samples fused into one op chain ----
              # NOTE: accumulation groups in one PSUM bank must be
              # contiguous (t-outer) — interleaving (k-outer) corrupts.
              gpb = psg.tile([128, SPC * TCH, 32], dt.float32, space="PSUM",
                             tag="gps")
              for s in range(SPC):
                for t in range(TCH):
                    for k in range(C_K):
                        nc.tensor.matmul(
                            out=gpb[:, TCH * s + t, :],
                            lhsT=xts[s][:, k, 128 * t:128 * (t + 1)],
                            rhs=gwts[s][:, k, :],
                            start=(k == 0), stop=(k == C_K - 1))
              # hi+lo sums: clean and noise logits [128, (s,t,e)]
              glo = gp.tile([128, SPC * TCH, 16], dt.float32, tag="glo")
              nc.vector.tensor_copy(glo[:], gpb[:, :, 16:32])
              lgc = gp.tile([128, SPC * TCH, E], dt.float32, tag="lgc")
              nc.vector.tensor_tensor(out=lgc[:], in0=gpb[:, :, 0:8],
                                      in1=glo[:, :, 0:8], op=ALU.add)
              lgn = gp.tile([128, SPC * TCH, E], dt.float32, tag="lgn")
              nc.vector.tensor_tensor(out=lgn[:], in0=gpb[:, :, 8:16],
                                      in1=glo[:, :, 8:16], op=ALU.add)
              # noise: eps * (softplus(lgn) + 0.01), softplus via exp/ln
              ex = gp.tile([128, SPC * TCH * E], dt.float32, tag="ex")
              nc.scalar.activation(ex[:], lgn[:].rearrange("p t e -> p (t e)"),
                                   AF.Exp)
              sp = gp.tile([128, SPC * TCH * E], dt.float32, tag="sp")
              nc.scalar.activation(sp[:], ex[:], AF.Ln, bias=1.0)
              spp = gp.tile([128, SPC * TCH * E], dt.float32, tag="spp")
              nc.vector.tensor_scalar_add(spp[:], sp[:], 0.01)
              nt = gp.tile([128, SPC * TCH * E], dt.float32, tag="nt")
              nc.vector.tensor_tensor(
                  out=nt[:], in0=spp[:],
                  in1=eps_both[:].rearrange("p s e -> p (s e)"), op=ALU.mult)
              # reduce over token chunks t, keep (s, e)
              rn = gp.tile([128, SPC * E], dt.float32, tag="rn")
              nc.vector.tensor_reduce(
                  out=rn[:], in_=nt[:].rearrange("p (s t e) -> p s e t",
                                                 s=SPC, t=TCH),
                  axis=mybir.AxisListType.X, op=ALU.add)
              rc = gp.tile([128, SPC * E], dt.float32, tag="rc")
              nc.vector.tensor_reduce(
                  out=rc[:], in_=lgc[:].rearrange("p (s t) e -> p s e t", s=SPC),
                  axis=mybir.AxisListType.X, op=ALU.add)
              tot = gp.tile([128, SPC * E], dt.float32, tag="tot")
              nc.vector.tensor_add(tot[:], rn[:], rc[:])
              # ews broadcast to all 128 partitions in one matmul:
              # out[m, (s,e)] = sum_p ones[p, m] * tot[p, (s,e)]
              b_ps = pst.tile([128, SPC * E], dt.float32, space="PSUM", tag="tps")
              nc.tensor.matmul(out=b_ps[:], lhsT=onesq[:], rhs=tot[:],
                               start=True, stop=True)
              ewsb = gp.tile([128, SPC * E], dt.float32, tag="ewsb")
              nc.vector.tensor_copy(ewsb[:], b_ps[:])

              states = []
              for s in range(SPC):
                mx = gp.tile([128, 8], dt.float32, tag="mx")
                mi = gp.tile([128, 8], dt.uint32, tag="mi")
                nc.vector.max_with_indices(mx[:], mi[:], ewsb[:, E * s:E * (s + 1)])

                # ---- top-2 expert weight gathers (gate folded in copy j) ----
                wts = []
                for j in range(TOPK):
                    idxf = gp.tile([128, 1], dt.float32, tag=f"idxf{j}")
                    nc.vector.tensor_copy(idxf[:], mi[:, j:j + 1])
                    rowf = gp.tile([128, 1], dt.float32, tag=f"rowf{j}")
                    if j == 0:
                        nc.vector.tensor_scalar(out=rowf[:], in0=idxf[:],
                                                scalar1=128.0, scalar2=None,
                                                op0=ALU.mult)
                    else:
                        nc.vector.tensor_scalar(out=rowf[:], in0=idxf[:],
                                                scalar1=128.0,
                                                scalar2=float(j * E * 128),
                                                op0=ALU.mult, op1=ALU.add)
                    nc.vector.tensor_add(rowf[:], rowf[:], iota_f[:])
                    gi = gp.tile([128, 1], dt.uint32, tag=f"gi{j}")
                    nc.vector.tensor_copy(gi[:], rowf[:])
                    wt = wtp.tile([128, PCK], dt.float8e4, tag=f"wt{j}", name=f"wt{j}")
                    nc.gpsimd.indirect_dma_start(
                        out=wt[:], out_offset=None, in_=wp_d[:],
                        in_offset=bass.IndirectOffsetOnAxis(ap=gi[:, :1], axis=0))
                    wts.append(wt)
                states.append(wts)

              # ---- expert phase ----
              for s in range(SPC):
                xt16, xt8, wts = xts[s], x8s[s], states[s]
                hTs = []
                for j in range(TOPK):
                    wt = wts[j]
                    b32 = gp.tile([128, 2], dt.float32, tag=f"b32_{j}")
                    nc.vector.tensor_copy(b32[:], wt[:, 3072:3074])
                    hT = hp.tile([128, 2, N], dt.float8e4, tag=f"hT{j}", name=f"hT{j}")
                    for n in range(NT):
                        for m in range(2):
                            f1p = psf.tile([128, 512], dt.float32, space="PSUM",
                                           tag="f1p")
                            for i in range(KP):
                                base = 768 * m + 256 * i
                                nc.tensor.matmul(
                                    out=f1p[:],
                                    lhsT=wt[:, base:base + 256]
                                        .rearrange("p (j m) -> p j m", j=2),
                                    rhs=xt8[:, 2 * i:2 * i + 2,
                                            512 * n:512 * (n + 1)],
                                    start=(i == 0), stop=(i == KP - 1),
                                    perf_mode=PM.DoubleRow)
                            nc.scalar.activation(
                                hT[:, m, 512 * n:512 * (n + 1)], f1p[:],
                                AF.Gelu, bias=b32[:, m:m + 1])
                    hTs.append(hT)

                # ---- fc2 (+bias via h ones-row) + residual, f16 out ----
                yst = yp.tile([128, C_K, N], dt.float16, tag="yst", name="yst")
                for ci in range(C_K):
                    for n in range(NT):
                        dr = _DRAIN[ci * NT + n]
                        yps = psy.tile([128, 512], dt.float32, space="PSUM",
                                       tag="yps")
                        for j in range(TOPK):
                            base = 1536 + 256 * ci
                            nc.tensor.matmul(
                                out=yps[:],
                                lhsT=wts[j][:, base:base + 256]
                                    .rearrange("p (j m) -> p j m", j=2),
                                rhs=hTs[j][:, :, 512 * n:512 * (n + 1)],
                                start=(j == 0), stop=(dr == "dve" and j == TOPK - 1),
                                perf_mode=PM.DoubleRow)
                        out_sl = yst[:, ci, 512 * n:512 * (n + 1)]
                        in_sl = xt16[:, ci, 512 * n:512 * (n + 1)]
                        if dr == "dve":
                            nc.vector.tensor_tensor(out=out_sl, in0=yps[:],
                                                    in1=in_sl, op=ALU.add)
                        else:
                            nc.tensor.matmul(out=yps[:], lhsT=id128[:],
                                             rhs=in_sl, start=False, stop=True)
                            nc.scalar.activation(out_sl, yps[:], AF.Copy)
                    if ci % 2 == 1:
                        nc.sync.dma_start(y_d[s, :, ci - 1:ci + 1, :],
                                          yst[:, ci - 1:ci + 1, :])

    nc.compile()
    _cache[key] = nc
    return nc


def _prep_inputs(x, task_ids, eps, gate_w, fc1_w, fc1_b, fc2_w, fc2_b):
    x = np.asarray(x, f32)
    task_ids = np.asarray(task_ids).astype(np.int64)
    eps = np.asarray(eps, f32)
    gate_w = np.asarray(gate_w, f32)
    f1w = np.asarray(fc1_w, f32)
    f1b = np.asarray(fc1_b, f32)
    f2w = np.asarray(fc2_w, f32)
    f2b = np.asarray(fc2_b, f32)

    # xT tiles [B, 128, C_K, N]
    xt16 = np.ascontiguousarray(
        x.reshape(B, N, C_K, 128).transpose(0, 3, 2, 1)).astype(f16)
    xt8 = xt16.astype(f8)

    # eps [B, 128, (t, e)]
    eps_dev = np.ascontiguousarray(
        eps.reshape(B, TCH, 128, E).transpose(0, 2, 1, 3)
    ).reshape(B, 128, TCH * E)

    # gate weights split f16 hi/lo: [B, 128, C_K, 32]
    gws = gate_w[task_ids]                       # [B, C, 16]
    g_hi = gws.astype(f16).astype(f32)
    g_lo = (gws - g_hi).astype(f16)
    cat = np.concatenate([g_hi.astype(f16), g_lo], axis=2)   # [B, C, 32]
    gw_dev = np.ascontiguousarray(
        cat.reshape(B, C_K, 128, 32).transpose(0, 2, 1, 3))

    # packed weights [2, E, 128, PCK] fp8; fc1 in DoubleRow (kpair, jj) order
    wp = np.zeros((2, E, 128, PCK), f32)
    a = f1w.reshape(E, H, C_K, 128).transpose(0, 3, 2, 1)    # [E, p, k, h]
    akj = a.reshape(E, 128, KP, 2, H)                        # [E, p, kp, jj, h]
    wp[:, :, :, 0:768] = akj[..., 0:128].reshape(E, 128, 768)
    m1 = np.zeros((E, 128, KP, 2, 128), f32)
    m1[..., 0:64] = akj[..., 128:192]
    wp[:, :, :, 768:1536] = m1.reshape(E, 128, 768)
    b0 = f2w.reshape(E, C_K, 128, H).transpose(0, 3, 1, 2)   # [E, h, ci, m]
    f2blk = np.zeros((E, 128, C_K, 2, 128), f32)
    f2blk[:, :, :, 0, :] = b0[:, 0:128]
    f2blk[:, 0:64, :, 1, :] = b0[:, 128:192]
    f2blk[:, 64, :, 1, :] = f2b.reshape(E, C_K, 128)
    for gidx, g in enumerate((G1, G2)):
        wp[gidx, :, :, 1536:3072] = (f2blk * g).reshape(E, 128, 1536)
    wp[:, :, :, 3072] = f1b[:, 0:128]
    bias1 = np.zeros((E, 128), f32)
    bias1[:, 0:64] = f1b[:, 128:192]
    bias1[:, 64] = ZSTAR
    wp[:, :, :, 3073] = bias1
    wpack = wp.reshape(2 * E * 128, PCK).astype(f8)

    id128 = np.eye(128, dtype=f16)

    in_maps = []
    for c in range(NCORES):
        sl = slice(SPC * c, SPC * (c + 1))
        in_maps.append({
            "xt16": xt16[sl], "xt8": xt8[sl],
            "gw": gw_dev[sl].astype(f16), "epsd": eps_dev[sl],
            "id128": id128, "wpack": wpack,
        })
    return in_maps


def kernel(x, task_ids, eps, gate_w, fc1_w, fc1_b, fc2_w, fc2_b, _trace=False):
    nc = _build()
    in_maps = _prep_inputs(x, task_ids, eps, gate_w, fc1_w, fc1_b, fc2_w, fc2_b)
    res = run_bass_kernel_spmd(nc, in_maps, list(range(NCORES)), trace=_trace)
    yt = np.concatenate([res.results[c]["y"] for c in range(NCORES)], axis=0)
    # [B, 128, C_K, N] -> [B, N, C]
    out = np.ascontiguousarray(
        yt.astype(f32).transpose(0, 3, 2, 1)).reshape(B, N, C)
    kernel.last_results = res
    return out


# revision 19
# speedup vs baseline: 1.0722x; 1.0722x over previous
"""MoE block (B=16,N=1024,C=768,E=8,H=192,D=4,K=2) on 8 NeuronCores.

Data-parallel over B (2 samples/core). Per sample:
  - noisy gating in split-f16 (hi+lo gate weights, f16 x) with tokens on
    partitions -> tiny matmuls; ews reduced on-chip; top-2 via max8.
    For K=2 the scaled-softmax gates are constants sigmoid(1)/1-sigmoid(1)
    (scaled = [1,0] always), so gates are folded into pre-scaled fc2 weight
    copies on the host.
  - one fp8 row-gather per selected expert (fc1+fc2+bias packed in DoubleRow
    interleave), fp8 DoubleRow fc1 and fc2 (0.5 cyc/row), exact-Gelu with
    per-partition bias (incl. a gelu(z)=1 row that feeds the fc2 bias through
    the matmul), residual added either fused in the DVE drain or via an f16
    identity matmul into the fc2 PSUM group with the drain copy on Act,
    f16 output.

Host prep is pure value-preserving re-layout: transpose, dtype split
(f16 hi/lo, fp8), index-gather of gate_w by task_ids, weight packing with
the constant top-2 gate folded in.

Layouts shipped from host (per sample):
  xt16  [128, 6, 1024] f16   xT (c%128 on partitions, c//128 chunks)
  xt8   [128, 6, 1024] f8    same, fp8 (fc1 rhs; DoubleRow pairs = chunk pairs)
  gw    [128, 6, 32]   f16   gate weights: clean_hi|noise_hi|clean_lo|noise_lo
  epsd  [128, 64]      f32   eps[(t,p),e] -> [p, (t,e)]
  id128 [128, 128]     f16   identity (PE residual add)
  wpack [2*E*128, 3080] f8   per (gate-copy, expert) packed rows:
        [0:768)    fc1 m0 (kpair3, jj2, h0:128)
        [768:1536) fc1 m1 (kpair3, jj2, h128:192|zeros)
        [1536:3072) fc2 (cchunk6, jj2, c128)*gate (+bias row at p=64,jj=1)
        [3072]     fc1 bias h0:128   [3073] fc1 bias h128:192 | z*(gelu->1) | 0
"""
import numpy as np
import ml_dtypes

import concourse.bass as bass
import concourse.mybir as mybir
import concourse.tile as tile
from concourse import bacc
from concourse.bass_utils import run_bass_kernel_spmd

f16 = np.float16
f8 = ml_dtypes.float8_e4m3
f32 = np.float32
AF = mybir.ActivationFunctionType
ALU = mybir.AluOpType
PM = mybir.MatmulPerfMode
dt = mybir.dt

B, N, C = 16, 1024, 768
E, H, D, TOPK = 8, 192, 4, 2
NCORES = 8
SPC = B // NCORES          # samples per core = 2
C_K = C // 128             # 6 channel chunks
KP = C_K // 2              # 3 DoubleRow k-pairs
TCH = N // 128             # 8 token chunks
NT = N // 512              # 2 big n-chunks
PCK = 3080                 # packed row bytes (fp8)
G1 = float(1.0 / (1.0 + np.exp(-1.0)))
G2 = 1.0 - G1
ZSTAR = 1.125              # f8-exact; gelu(1.125)=0.978 -> f8 rounds to 1.0

# drain engine per (ci, n) chunk: DVE does fused residual adds; Act does
# pure copies (residual added by PE identity matmul into PSUM). GPSIMD has
# no PSUM port, so Pool cannot help here.
_DRAIN = ["dve", "act", "dve", "dve", "dve", "dve"] * 2

_cache = {}


def _build(reps=1):
    key = ("nc", reps)
    if key in _cache:
        return _cache[key]
    nc = bacc.Bacc("TRN2", target_bir_lowering=False, debug=False,
                   num_devices=NCORES)

    x16_d = nc.dram_tensor("xt16", [SPC, 128, C_K, N], dt.float16, kind="ExternalInput").ap()
    x8_d = nc.dram_tensor("xt8", [SPC, 128, C_K, N], dt.float8e4, kind="ExternalInput").ap()
    gw_d = nc.dram_tensor("gw", [SPC, 128, C_K, 32], dt.float16, kind="ExternalInput").ap()
    ep_d = nc.dram_tensor("epsd", [SPC, 128, TCH * E], dt.float32, kind="ExternalInput").ap()
    id_d = nc.dram_tensor("id128", [128, 128], dt.float16, kind="ExternalInput").ap()
    wp_d = nc.dram_tensor("wpack", [2 * E * 128, PCK], dt.float8e4, kind="ExternalInput").ap()
    y_d = nc.dram_tensor("y", [SPC, 128, C_K, N], dt.float16, kind="ExternalOutput").ap()

    with tile.TileContext(nc) as tc:
        with tc.tile_pool(name="const", bufs=1) as cp, \
             tc.tile_pool(name="xt", bufs=2) as xtp, \
             tc.tile_pool(name="gate", bufs=2) as gp, \
             tc.tile_pool(name="wt", bufs=2) as wtp, \
             tc.tile_pool(name="h", bufs=2) as hp, \
             tc.tile_pool(name="yout", bufs=2) as yp, \
             tc.tile_pool(name="ps_g", bufs=1, space="PSUM") as psg, \
             tc.tile_pool(name="ps_t", bufs=1, space="PSUM") as pst, \
             tc.tile_pool(name="ps_f1", bufs=3, space="PSUM") as psf, \
             tc.tile_pool(name="ps_y", bufs=2, space="PSUM") as psy:

            # constants (no DMA except id128)
            iota_i = cp.tile([128, 1], dt.int32, tag="iota_i")
            nc.gpsimd.iota(iota_i[:], pattern=[[0, 1]], base=0, channel_multiplier=1)
            iota_f = cp.tile([128, 1], dt.float32, tag="iota_f")
            nc.vector.tensor_copy(iota_f[:], iota_i[:])
            onesq = cp.tile([128, 128], dt.float32, tag="onesq")
            nc.vector.memset(onesq[:], 1.0)
            id128 = cp.tile([128, 128], dt.float16, tag="id128")
            # explicitly pre-load the exp+ln act table (set 6 =
            # natural_log_exp_and_others) so the softplus chain (Exp then
            # Ln, possibly interleaved by the scheduler) needs no implicit
            # table loads; the gelu table loads once later.
            nc.scalar.add_instruction(mybir.InstLoadActFuncSet(
                name=nc.get_next_instruction_name(), ins=[], outs=[],
                act_func_set_id=6))

            for rep in range(reps):
              # ---- loads: small tensors first, x chunks, fp8 x last ----
              gwts, xts = [], []
              eps_both = gp.tile([128, SPC, TCH * E], dt.float32, tag="epsb")
              for s in range(SPC):
                  gwt = gp.tile([128, C_K, 32], dt.float16, tag="gwt")
                  nc.sync.dma_start(gwt[:], gw_d[s])
                  gwts.append(gwt)
              for s in range(SPC):
                  xt16 = xtp.tile([128, C_K, N], dt.float16, tag="xt16",
                                  name="xt16")
                  for k in range(C_K):
                      nc.sync.dma_start(xt16[:, k, :], x16_d[s, :, k, :])
                  xts.append(xt16)
              for s in range(SPC):
                  nc.sync.dma_start(eps_both[:, s, :], ep_d[s])
              x8s = []
              for s in range(SPC):
                  xt8 = xtp.tile([128, C_K, N], dt.float8e4, tag="xt8",
                                 name="xt8")
                  nc.sync.dma_start(xt8[:], x8_d[s])
                  x8s.append(xt8)
              if rep == 0:
                  nc.sync.dma_start(id128[:], id_d[:, :])

              # ---- gating, both samples fused into one op chain ----
              # NOTE: accumulation groups in one PSUM bank must be
              # contiguous (t-outer) — interleaving (k-outer) corrupts.
              gpb = psg.tile([128, SPC * TCH, 32], dt.float32, space="PSUM",
                             tag="gps")
              for s in range(SPC):
                for t in range(TCH):
                    for k in range(C_K):
                        nc.tensor.matmul(
                            out=gpb[:, TCH * s + t, :],
                            lhsT=xts[s][:, k, 128 * t:128 * (t + 1)],
                            rhs=gwts[s][:, k, :],
                            start=(k == 0), stop=(k == C_K - 1))
              # hi+lo sums: clean and noise logits [128, (s,t,e)]
              glo = gp.tile([128, SPC * TCH, 16], dt.float32, tag="glo")
              nc.vector.tensor_copy(glo[:], gpb[:, :, 16:32])
              lgc = gp.tile([128, SPC * TCH, E], dt.float32, tag="lgc")
              nc.vector.tensor_tensor(out=lgc[:], in0=gpb[:, :, 0:8],
                                      in1=glo[:, :, 0:8], op=ALU.add)
              lgn = gp.tile([128, SPC * TCH, E], dt.float32, tag="lgn")
              nc.vector.tensor_tensor(out=lgn[:], in0=gpb[:, :, 8:16],
                                      in1=glo[:, :, 8:16], op=ALU.add)
              # noise: eps * (softplus(lgn) + 0.01), softplus via exp/ln
              ex = gp.tile([128, SPC * TCH * E], dt.float32, tag="ex")
              nc.scalar.activation(ex[:], lgn[:].rearrange("p t e -> p (t e)"),
                                   AF.Exp)
              sp = gp.tile([128, SPC * TCH * E], dt.float32, tag="sp")
              nc.scalar.activation(sp[:], ex[:], AF.Ln, bias=1.0)
              spp = gp.tile([128, SPC * TCH * E], dt.float32, tag="spp")
              nc.vector.tensor_scalar_add(spp[:], sp[:], 0.01)
              nt = gp.tile([128, SPC * TCH * E], dt.float32, tag="nt")
              nc.vector.tensor_tensor(
                  out=nt[:], in0=spp[:],
                  in1=eps_both[:].rearrange("p s e -> p (s e)"), op=ALU.mult)
              # reduce over token chunks t, keep (s, e)
              rn = gp.tile([128, SPC * E], dt.float32, tag="rn")
              nc.vector.tensor_reduce(
                  out=rn[:], in_=nt[:].rearrange("p (s t e) -> p s e t",
                                                 s=SPC, t=TCH),
                  axis=mybir.AxisListType.X, op=ALU.add)
              rc = gp.tile([128, SPC * E], dt.float32, tag="rc")
              nc.vector.tensor_reduce(
                  out=rc[:], in_=lgc[:].rearrange("p (s t) e -> p s e t", s=SPC),
                  axis=mybir.AxisListType.X, op=ALU.add)
              tot = gp.tile([128, SPC * E], dt.float32, tag="tot")
              nc.vector.tensor_add(tot[:], rn[:], rc[:])
              # ews broadcast to all 128 partitions in one matmul:
              # out[m, (s,e)] = sum_p ones[p, m] * tot[p, (s,e)]
              b_ps = pst.tile([128, SPC * E], dt.float32, space="PSUM", tag="tps")
              nc.tensor.matmul(out=b_ps[:], lhsT=onesq[:], rhs=tot[:],
                               start=True, stop=True)
              ewsb = gp.tile([128, SPC * E], dt.float32, tag="ewsb")
              nc.vector.tensor_copy(ewsb[:], b_ps[:])

              states = []
              for s in range(SPC):
                mx = gp.tile([128, 8], dt.float32, tag="mx")
                mi = gp.tile([128, 8], dt.uint32, tag="mi")
                nc.vector.max_with_indices(mx[:], mi[:], ewsb[:, E * s:E * (s + 1)])

                # ---- top-2 expert weight gathers (gate folded in copy j) ----
                wts = []
                for j in range(TOPK):
                    idxf = gp.tile([128, 1], dt.float32, tag=f"idxf{j}")
                    nc.vector.tensor_copy(idxf[:], mi[:, j:j + 1])
                    rowf = gp.tile([128, 1], dt.float32, tag=f"rowf{j}")
                    if j == 0:
                        nc.vector.tensor_scalar(out=rowf[:], in0=idxf[:],
                                                scalar1=128.0, scalar2=None,
                                                op0=ALU.mult)
                    else:
                        nc.vector.tensor_scalar(out=rowf[:], in0=idxf[:],
                                                scalar1=128.0,
                                                scalar2=float(j * E * 128),
                                                op0=ALU.mult, op1=ALU.add)
                    nc.vector.tensor_add(rowf[:], rowf[:], iota_f[:])
                    gi = gp.tile([128, 1], dt.uint32, tag=f"gi{j}")
                    nc.vector.tensor_copy(gi[:], rowf[:])
                    wt = wtp.tile([128, PCK], dt.float8e4, tag=f"wt{j}", name=f"wt{j}")
                    nc.gpsimd.indirect_dma_start(
                        out=wt[:], out_offset=None, in_=wp_d[:],
                        in_offset=bass.IndirectOffsetOnAxis(ap=gi[:, :1], axis=0))
                    wts.append(wt)
                states.append(wts)

              # ---- expert phase: n-outer so fc2/drains start after the
              # first half's gelus instead of after all fc1 ----
              for s in range(SPC):
                xt16, xt8, wts = xts[s], x8s[s], states[s]
                b32s, hTs = [], []
                for j in range(TOPK):
                    b32 = gp.tile([128, 2], dt.float32, tag=f"b32_{j}")
                    nc.vector.tensor_copy(b32[:], wts[j][:, 3072:3074])
                    b32s.append(b32)
                    hTs.append(hp.tile([128, 2, N], dt.float8e4,
                                       tag=f"hT{j}", name=f"hT{j}"))
                yst = yp.tile([128, C_K, N], dt.float16, tag="yst", name="yst")
                for n in range(NT):
                    nsl = slice(512 * n, 512 * (n + 1))
                    for j in range(TOPK):
                        for m in range(2):
                            f1p = psf.tile([128, 512], dt.float32, space="PSUM",
                                           tag="f1p")
                            for i in range(KP):
                                base = 768 * m + 256 * i
                                nc.tensor.matmul(
                                    out=f1p[:],
                                    lhsT=wts[j][:, base:base + 256]
                                        .rearrange("p (j m) -> p j m", j=2),
                                    rhs=xt8[:, 2 * i:2 * i + 2, nsl],
                                    start=(i == 0), stop=(i == KP - 1),
                                    perf_mode=PM.DoubleRow)
                            nc.scalar.activation(
                                hTs[j][:, m, nsl], f1p[:],
                                AF.Gelu, bias=b32s[j][:, m:m + 1])
                    # fc2 (+bias via h ones-row) + residual for this half
                    for ci in range(C_K):
                        dr = _DRAIN[ci * NT + n]
                        yps = psy.tile([128, 512], dt.float32, space="PSUM",
                                       tag="yps")
                        for j in range(TOPK):
                            base = 1536 + 256 * ci
                            nc.tensor.matmul(
                                out=yps[:],
                                lhsT=wts[j][:, base:base + 256]
                                    .rearrange("p (j m) -> p j m", j=2),
                                rhs=hTs[j][:, :, nsl],
                                start=(j == 0), stop=(dr == "dve" and j == TOPK - 1),
                                perf_mode=PM.DoubleRow)
                        out_sl = yst[:, ci, nsl]
                        in_sl = xt16[:, ci, nsl]
                        if dr == "dve":
                            nc.vector.tensor_tensor(out=out_sl, in0=yps[:],
                                                    in1=in_sl, op=ALU.add)
                        else:
                            nc.tensor.matmul(out=yps[:], lhsT=id128[:],
                                             rhs=in_sl, start=False, stop=True)
                            nc.scalar.activation(out_sl, yps[:], AF.Copy)
                        if ci % 3 == 2:
                            nc.sync.dma_start(
                                y_d[s, :, ci - 2:ci + 1, nsl],
                                yst[:, ci - 2:ci + 1, nsl])

    nc.compile()
    _cache[key] = nc
    return nc


def _prep_inputs(x, task_ids, eps, gate_w, fc1_w, fc1_b, fc2_w, fc2_b):
    x = np.asarray(x, f32)
    task_ids = np.asarray(task_ids).astype(np.int64)
    eps = np.asarray(eps, f32)
    gate_w = np.asarray(gate_w, f32)
    f1w = np.asarray(fc1_w, f32)
    f1b = np.asarray(fc1_b, f32)
    f2w = np.asarray(fc2_w, f32)
    f2b = np.asarray(fc2_b, f32)

    # xT tiles [B, 128, C_K, N]
    xt16 = np.ascontiguousarray(
        x.reshape(B, N, C_K, 128).transpose(0, 3, 2, 1)).astype(f16)
    xt8 = xt16.astype(f8)

    # eps [B, 128, (t, e)]
    eps_dev = np.ascontiguousarray(
        eps.reshape(B, TCH, 128, E).transpose(0, 2, 1, 3)
    ).reshape(B, 128, TCH * E)

    # gate weights split f16 hi/lo: [B, 128, C_K, 32]
    gws = gate_w[task_ids]                       # [B, C, 16]
    g_hi = gws.astype(f16).astype(f32)
    g_lo = (gws - g_hi).astype(f16)
    cat = np.concatenate([g_hi.astype(f16), g_lo], axis=2)   # [B, C, 32]
    gw_dev = np.ascontiguousarray(
        cat.reshape(B, C_K, 128, 32).transpose(0, 2, 1, 3))

    # packed weights [2, E, 128, PCK] fp8; fc1 in DoubleRow (kpair, jj) order
    wp = np.zeros((2, E, 128, PCK), f32)
    a = f1w.reshape(E, H, C_K, 128).transpose(0, 3, 2, 1)    # [E, p, k, h]
    akj = a.reshape(E, 128, KP, 2, H)                        # [E, p, kp, jj, h]
    wp[:, :, :, 0:768] = akj[..., 0:128].reshape(E, 128, 768)
    m1 = np.zeros((E, 128, KP, 2, 128), f32)
    m1[..., 0:64] = akj[..., 128:192]
    wp[:, :, :, 768:1536] = m1.reshape(E, 128, 768)
    b0 = f2w.reshape(E, C_K, 128, H).transpose(0, 3, 1, 2)   # [E, h, ci, m]
    f2blk = np.zeros((E, 128, C_K, 2, 128), f32)
    f2blk[:, :, :, 0, :] = b0[:, 0:128]
    f2blk[:, 0:64, :, 1, :] = b0[:, 128:192]
    f2blk[:, 64, :, 1, :] = f2b.reshape(E, C_K, 128)
    for gidx, g in enumerate((G1, G2)):
        wp[gidx, :, :, 1536:3072] = (f2blk * g).reshape(E, 128, 1536)
    wp[:, :, :, 3072] = f1b[:, 0:128]
    bias1 = np.zeros((E, 128), f32)
    bias1[:, 0:64] = f1b[:, 128:192]
    bias1[:, 64] = ZSTAR
    wp[:, :, :, 3073] = bias1
    wpack = wp.reshape(2 * E * 128, PCK).astype(f8)

    id128 = np.eye(128, dtype=f16)

    in_maps = []
    for c in range(NCORES):
        sl = slice(SPC * c, SPC * (c + 1))
        in_maps.append({
            "xt16": xt16[sl], "xt8": xt8[sl],
            "gw": gw_dev[sl].astype(f16), "epsd": eps_dev[sl],
            "id128": id128, "wpack": wpack,
        })
    return in_maps


def kernel(x, task_ids, eps, gate_w, fc1_w, fc1_b, fc2_w, fc2_b, _trace=False):
    nc = _build()
    in_maps = _prep_inputs(x, task_ids, eps, gate_w, fc1_w, fc1_b, fc2_w, fc2_b)
    res = run_bass_kernel_spmd(nc, in_maps, list(range(NCORES)), trace=_trace)
    yt = np.concatenate([res.results[c]["y"] for c in range(NCORES)], axis=0)
    # [B, 128, C_K, N] -> [B, N, C]
    out = np.ascontiguousarray(
        yt.astype(f32).transpose(0, 3, 2, 1)).reshape(B, N, C)
    kernel.last_results = res
    return out


# revision 22
# speedup vs baseline: 1.0797x; 1.0069x over previous
"""MoE block (B=16,N=1024,C=768,E=8,H=192,D=4,K=2) on 8 NeuronCores.

Data-parallel over B (2 samples/core). Per sample:
  - noisy gating in split-f16 (hi+lo gate weights, f16 x) with tokens on
    partitions -> tiny matmuls; ews reduced on-chip; top-2 via max8.
    For K=2 the scaled-softmax gates are constants sigmoid(1)/1-sigmoid(1)
    (scaled = [1,0] always), so gates are folded into pre-scaled fc2 weight
    copies on the host.
  - one fp8 row-gather per selected expert (fc1+fc2+bias packed in DoubleRow
    interleave), fp8 DoubleRow fc1 and fc2 (0.5 cyc/row), exact-Gelu with
    per-partition bias (incl. a gelu(z)=1 row that feeds the fc2 bias through
    the matmul), residual added either fused in the DVE drain or via an f16
    identity matmul into the fc2 PSUM group with the drain copy on Act,
    f16 output.

Host prep is pure value-preserving re-layout: transpose, dtype split
(f16 hi/lo, fp8), index-gather of gate_w by task_ids, weight packing with
the constant top-2 gate folded in.

Layouts shipped from host (per sample):
  xt16  [128, 6, 1024] f16   xT (c%128 on partitions, c//128 chunks)
  xt8   [128, 6, 1024] f8    same, fp8 (fc1 rhs; DoubleRow pairs = chunk pairs)
  gw    [128, 6, 32]   f16   gate weights: clean_hi|noise_hi|clean_lo|noise_lo
  epsd  [128, 64]      f32   eps[(t,p),e] -> [p, (t,e)]
  id128 [128, 128]     f16   identity (PE residual add)
  wpack [2*E*128, 3080] f8   per (gate-copy, expert) packed rows:
        [0:768)    fc1 m0 (kpair3, jj2, h0:128)
        [768:1536) fc1 m1 (kpair3, jj2, h128:192|zeros)
        [1536:3072) fc2 (cchunk6, jj2, c128)*gate (+bias row at p=64,jj=1)
        [3072]     fc1 bias h0:128   [3073] fc1 bias h128:192 | z*(gelu->1) | 0
"""
import numpy as np
import ml_dtypes

import concourse.bass as bass
import concourse.mybir as mybir
import concourse.tile as tile
from concourse import bacc
from concourse.bass_utils import run_bass_kernel_spmd

f16 = np.float16
f8 = ml_dtypes.float8_e4m3
f32 = np.float32
AF = mybir.ActivationFunctionType
ALU = mybir.AluOpType
PM = mybir.MatmulPerfMode
dt = mybir.dt

B, N, C = 16, 1024, 768
E, H, D, TOPK = 8, 192, 4, 2
NCORES = 8
SPC = B // NCORES          # samples per core = 2
C_K = C // 128             # 6 channel chunks
KP = C_K // 2              # 3 DoubleRow k-pairs
TCH = N // 128             # 8 token chunks
NT = N // 512              # 2 big n-chunks
PCK = 3080                 # packed row bytes (fp8)
G1 = float(1.0 / (1.0 + np.exp(-1.0)))
G2 = 1.0 - G1
ZSTAR = 1.125              # f8-exact; gelu(1.125)=0.978 -> f8 rounds to 1.0

# drain engine per (ci, n) chunk: DVE does fused residual adds; Act does
# pure copies (residual added by PE identity matmul into PSUM). GPSIMD has
# no PSUM port, so Pool cannot help here.
_DRAIN = ["dve", "act", "dve", "dve", "dve", "dve"] * 2

_cache = {}


def _build(reps=1):
    key = ("nc", reps)
    if key in _cache:
        return _cache[key]
    nc = bacc.Bacc("TRN2", target_bir_lowering=False, debug=False,
                   num_devices=NCORES)

    x16_d = nc.dram_tensor("xt16", [SPC, 128, C_K, N], dt.float16, kind="ExternalInput").ap()
    x8_d = nc.dram_tensor("xt8", [SPC, 128, C_K, N], dt.float8e4, kind="ExternalInput").ap()
    gw_d = nc.dram_tensor("gw", [SPC, 128, C_K, 32], dt.float16, kind="ExternalInput").ap()
    ep_d = nc.dram_tensor("epsd", [SPC, 128, TCH * E], dt.float32, kind="ExternalInput").ap()
    id_d = nc.dram_tensor("id128", [128, 128], dt.float16, kind="ExternalInput").ap()
    wp_d = nc.dram_tensor("wpack", [2 * E * 128, PCK], dt.float8e4, kind="ExternalInput").ap()
    y_d = nc.dram_tensor("y", [SPC, 128, C_K, N], dt.float16, kind="ExternalOutput").ap()

    with tile.TileContext(nc) as tc:
        with tc.tile_pool(name="const", bufs=1) as cp, \
             tc.tile_pool(name="xt", bufs=2) as xtp, \
             tc.tile_pool(name="gate", bufs=2) as gp, \
             tc.tile_pool(name="wt", bufs=2) as wtp, \
             tc.tile_pool(name="h", bufs=2) as hp, \
             tc.tile_pool(name="yout", bufs=2) as yp, \
             tc.tile_pool(name="ps_g", bufs=1, space="PSUM") as psg, \
             tc.tile_pool(name="ps_t", bufs=1, space="PSUM") as pst, \
             tc.tile_pool(name="ps_f1", bufs=3, space="PSUM") as psf, \
             tc.tile_pool(name="ps_y", bufs=2, space="PSUM") as psy:

            # constants (no DMA except id128)
            iota_i = cp.tile([128, 1], dt.int32, tag="iota_i")
            nc.gpsimd.iota(iota_i[:], pattern=[[0, 1]], base=0, channel_multiplier=1)
            iota_f = cp.tile([128, 1], dt.float32, tag="iota_f")
            nc.vector.tensor_copy(iota_f[:], iota_i[:])
            # col j = iota + j*1024 (row offset of gate-copy j in wpack)
            iota2 = cp.tile([128, 2], dt.float32, tag="iota2")
            nc.vector.tensor_copy(iota2[:, 0:1], iota_f[:])
            nc.vector.tensor_scalar_add(iota2[:, 1:2], iota_f[:],
                                        float(E * 128))
            onesq = cp.tile([128, 128], dt.float32, tag="onesq")
            nc.vector.memset(onesq[:], 1.0)
            id128 = cp.tile([128, 128], dt.float16, tag="id128")
            # explicitly pre-load the exp+ln act table (set 6 =
            # natural_log_exp_and_others) so the softplus chain (Exp then
            # Ln, possibly interleaved by the scheduler) needs no implicit
            # table loads; the gelu table loads once later.
            nc.scalar.add_instruction(mybir.InstLoadActFuncSet(
                name=nc.get_next_instruction_name(), ins=[], outs=[],
                act_func_set_id=6))

            for rep in range(reps):
              # ---- loads: small tensors first, x chunks, fp8 x last ----
              gwts, xts = [], []
              eps_both = gp.tile([128, SPC, TCH * E], dt.float32, tag="epsb")
              for s in range(SPC):
                  gwt = gp.tile([128, C_K, 32], dt.float16, tag="gwt")
                  nc.sync.dma_start(gwt[:], gw_d[s])
                  gwts.append(gwt)
              for s in range(SPC):
                  xt16 = xtp.tile([128, C_K, N], dt.float16, tag="xt16",
                                  name="xt16")
                  for k in range(C_K):
                      nc.sync.dma_start(xt16[:, k, :], x16_d[s, :, k, :])
                  xts.append(xt16)
              for s in range(SPC):
                  nc.sync.dma_start(eps_both[:, s, :], ep_d[s])
              x8s = []
              for s in range(SPC):
                  xt8 = xtp.tile([128, C_K, N], dt.float8e4, tag="xt8",
                                 name="xt8")
                  nc.sync.dma_start(xt8[:], x8_d[s])
                  x8s.append(xt8)
              if rep == 0:
                  nc.sync.dma_start(id128[:], id_d[:, :])

              # ---- gating, both samples fused into one op chain ----
              # NOTE: accumulation groups in one PSUM bank must be
              # contiguous (t-outer) — interleaving (k-outer) corrupts.
              gpb = psg.tile([128, SPC * TCH, 32], dt.float32, space="PSUM",
                             tag="gps")
              for s in range(SPC):
                for t in range(TCH):
                    for k in range(C_K):
                        nc.tensor.matmul(
                            out=gpb[:, TCH * s + t, :],
                            lhsT=xts[s][:, k, 128 * t:128 * (t + 1)],
                            rhs=gwts[s][:, k, :],
                            start=(k == 0), stop=(k == C_K - 1))
              # hi+lo sums: clean and noise logits [128, (s,t,e)]
              glo = gp.tile([128, SPC * TCH, 16], dt.float32, tag="glo")
              nc.scalar.activation(glo[:], gpb[:, :, 16:32], AF.Copy)
              lgc = gp.tile([128, SPC * TCH, E], dt.float32, tag="lgc")
              nc.vector.tensor_tensor(out=lgc[:], in0=gpb[:, :, 0:8],
                                      in1=glo[:, :, 0:8], op=ALU.add)
              lgn = gp.tile([128, SPC * TCH, E], dt.float32, tag="lgn")
              nc.vector.tensor_tensor(out=lgn[:], in0=gpb[:, :, 8:16],
                                      in1=glo[:, :, 8:16], op=ALU.add)
              # noise std: softplus(lgn) + 0.01 == Ln(s*exp(lgn) + s), s=e^.01
              ex = gp.tile([128, SPC * TCH * E], dt.float32, tag="ex")
              nc.scalar.activation(ex[:], lgn[:].rearrange("p t e -> p (t e)"),
                                   AF.Exp)
              S01 = float(np.exp(0.01))
              s01t = gp.tile([128, 1], dt.float32, tag="s01t")
              nc.vector.memset(s01t[:], S01)
              sp = gp.tile([128, SPC * TCH * E], dt.float32, tag="sp")
              nc.scalar.activation(sp[:], ex[:], AF.Ln, bias=s01t[:, 0:1],
                                   scale=S01)
              # pull the gelu act-table load ahead of the fc1 gelus: a dummy
              # gelu data-dependent on Ln so the scheduler keeps it here.
              dgel = gp.tile([1, 1], dt.float32, tag="dgel")
              nc.scalar.activation(dgel[:], sp[0:1, 0:1], AF.Gelu)
              nt = gp.tile([128, SPC * TCH * E], dt.float32, tag="nt")
              nc.vector.tensor_tensor(
                  out=nt[:], in0=sp[:],
                  in1=eps_both[:].rearrange("p s e -> p (s e)"), op=ALU.mult)
              ntc = gp.tile([128, SPC * TCH * E], dt.float32, tag="ntc")
              nc.vector.tensor_tensor(
                  out=ntc[:], in0=nt[:],
                  in1=lgc[:].rearrange("p t e -> p (t e)"), op=ALU.add)
              # reduce over token chunks t, keep (s, e)
              tot = gp.tile([128, SPC * E], dt.float32, tag="tot")
              nc.vector.tensor_reduce(
                  out=tot[:], in_=ntc[:].rearrange("p (s t e) -> p s e t",
                                                   s=SPC, t=TCH),
                  axis=mybir.AxisListType.X, op=ALU.add)
              # ews broadcast to all 128 partitions in one matmul:
              # out[m, (s,e)] = sum_p ones[p, m] * tot[p, (s,e)]
              b_ps = pst.tile([128, SPC * E], dt.float32, space="PSUM", tag="tps")
              nc.tensor.matmul(out=b_ps[:], lhsT=onesq[:], rhs=tot[:],
                               start=True, stop=True)
              ewsb = gp.tile([128, SPC * E], dt.float32, tag="ewsb")
              nc.vector.tensor_copy(ewsb[:], b_ps[:])

              states = []
              for s in range(SPC):
                mx = gp.tile([128, 8], dt.float32, tag="mx")
                mi = gp.tile([128, 8], dt.uint32, tag="mi")
                nc.vector.max_with_indices(mx[:], mi[:], ewsb[:, E * s:E * (s + 1)])

                # ---- top-2 expert weight gathers (gate folded in copy j) ----
                idxf = gp.tile([128, 2], dt.float32, tag="idxf")
                nc.vector.tensor_copy(idxf[:], mi[:, 0:2])
                rowf = gp.tile([128, 2], dt.float32, tag="rowf")
                nc.vector.tensor_scalar(out=rowf[:], in0=idxf[:],
                                        scalar1=128.0, scalar2=None,
                                        op0=ALU.mult)
                nc.vector.tensor_add(rowf[:], rowf[:], iota2[:])
                gi = gp.tile([128, 2], dt.uint32, tag="gi")
                nc.vector.tensor_copy(gi[:], rowf[:])
                wts = []
                for j in range(TOPK):
                    wt = wtp.tile([128, PCK], dt.float8e4, tag=f"wt{j}", name=f"wt{j}")
                    nc.gpsimd.indirect_dma_start(
                        out=wt[:], out_offset=None, in_=wp_d[:],
                        in_offset=bass.IndirectOffsetOnAxis(ap=gi[:, j:j + 1], axis=0))
                    wts.append(wt)
                states.append(wts)

              # ---- expert phase: n-outer so fc2/drains start after the
              # first half's gelus instead of after all fc1 ----
              for s in range(SPC):
                xt16, xt8, wts = xts[s], x8s[s], states[s]
                b32s, hTs = [], []
                for j in range(TOPK):
                    b32 = gp.tile([128, 2], dt.float32, tag=f"b32_{j}")
                    nc.vector.tensor_copy(b32[:], wts[j][:, 3072:3074])
                    b32s.append(b32)
                    hTs.append(hp.tile([128, 2, N], dt.float8e4,
                                       tag=f"hT{j}", name=f"hT{j}"))
                yst = yp.tile([128, C_K, N], dt.float16, tag="yst", name="yst")
                for n in range(NT):
                    nsl = slice(512 * n, 512 * (n + 1))
                    for j in range(TOPK):
                        for m in range(2):
                            f1p = psf.tile([128, 512], dt.float32, space="PSUM",
                                           tag="f1p")
                            for i in range(KP):
                                base = 768 * m + 256 * i
                                nc.tensor.matmul(
                                    out=f1p[:],
                                    lhsT=wts[j][:, base:base + 256]
                                        .rearrange("p (j m) -> p j m", j=2),
                                    rhs=xt8[:, 2 * i:2 * i + 2, nsl],
                                    start=(i == 0), stop=(i == KP - 1),
                                    perf_mode=PM.DoubleRow)
                            nc.scalar.activation(
                                hTs[j][:, m, nsl], f1p[:],
                                AF.Gelu, bias=b32s[j][:, m:m + 1])
                    # fc2 (+bias via h ones-row) + residual for this half
                    for ci in range(C_K):
                        dr = _DRAIN[ci * NT + n]
                        yps = psy.tile([128, 512], dt.float32, space="PSUM",
                                       tag="yps")
                        for j in range(TOPK):
                            base = 1536 + 256 * ci
                            nc.tensor.matmul(
                                out=yps[:],
                                lhsT=wts[j][:, base:base + 256]
                                    .rearrange("p (j m) -> p j m", j=2),
                                rhs=hTs[j][:, :, nsl],
                                start=(j == 0), stop=(dr == "dve" and j == TOPK - 1),
                                perf_mode=PM.DoubleRow)
                        out_sl = yst[:, ci, nsl]
                        in_sl = xt16[:, ci, nsl]
                        if dr == "dve":
                            nc.vector.tensor_tensor(out=out_sl, in0=yps[:],
                                                    in1=in_sl, op=ALU.add)
                        else:
                            nc.tensor.matmul(out=yps[:], lhsT=id128[:],
                                             rhs=in_sl, start=False, stop=True)
                            nc.scalar.activation(out_sl, yps[:], AF.Copy)
                        if ci % 3 == 2:
                            nc.sync.dma_start(
                                y_d[s, :, ci - 2:ci + 1, nsl],
                                yst[:, ci - 2:ci + 1, nsl])

    nc.compile()
    _cache[key] = nc
    return nc


def _prep_inputs(x, task_ids, eps, gate_w, fc1_w, fc1_b, fc2_w, fc2_b):
    x = np.asarray(x, f32)
    task_ids = np.asarray(task_ids).astype(np.int64)
    eps = np.asarray(eps, f32)
    gate_w = np.asarray(gate_w, f32)
    f1w = np.asarray(fc1_w, f32)
    f1b = np.asarray(fc1_b, f32)
    f2w = np.asarray(fc2_w, f32)
    f2b = np.asarray(fc2_b, f32)

    # xT tiles [B, 128, C_K, N]
    xt16 = np.ascontiguousarray(
        x.reshape(B, N, C_K, 128).transpose(0, 3, 2, 1)).astype(f16)
    xt8 = xt16.astype(f8)

    # eps [B, 128, (t, e)]
    eps_dev = np.ascontiguousarray(
        eps.reshape(B, TCH, 128, E).transpose(0, 2, 1, 3)
    ).reshape(B, 128, TCH * E)

    # gate weights split f16 hi/lo: [B, 128, C_K, 32]
    gws = gate_w[task_ids]                       # [B, C, 16]
    g_hi = gws.astype(f16).astype(f32)
    g_lo = (gws - g_hi).astype(f16)
    cat = np.concatenate([g_hi.astype(f16), g_lo], axis=2)   # [B, C, 32]
    gw_dev = np.ascontiguousarray(
        cat.reshape(B, C_K, 128, 32).transpose(0, 2, 1, 3))

    # packed weights [2, E, 128, PCK] fp8; fc1 in DoubleRow (kpair, jj) order
    wp = np.zeros((2, E, 128, PCK), f32)
    a = f1w.reshape(E, H, C_K, 128).transpose(0, 3, 2, 1)    # [E, p, k, h]
    akj = a.reshape(E, 128, KP, 2, H)                        # [E, p, kp, jj, h]
    wp[:, :, :, 0:768] = akj[..., 0:128].reshape(E, 128, 768)
    m1 = np.zeros((E, 128, KP, 2, 128), f32)
    m1[..., 0:64] = akj[..., 128:192]
    wp[:, :, :, 768:1536] = m1.reshape(E, 128, 768)
    b0 = f2w.reshape(E, C_K, 128, H).transpose(0, 3, 1, 2)   # [E, h, ci, m]
    f2blk = np.zeros((E, 128, C_K, 2, 128), f32)
    f2blk[:, :, :, 0, :] = b0[:, 0:128]
    f2blk[:, 0:64, :, 1, :] = b0[:, 128:192]
    f2blk[:, 64, :, 1, :] = f2b.reshape(E, C_K, 128)
    for gidx, g in enumerate((G1, G2)):
        wp[gidx, :, :, 1536:3072] = (f2blk * g).reshape(E, 128, 1536)
    wp[:, :, :, 3072] = f1b[:, 0:128]
    bias1 = np.zeros((E, 128), f32)
    bias1[:, 0:64] = f1b[:, 128:192]
    bias1[:, 64] = ZSTAR
    wp[:, :, :, 3073] = bias1
    wpack = wp.reshape(2 * E * 128, PCK).astype(f8)

    id128 = np.eye(128, dtype=f16)

    in_maps = []
    for c in range(NCORES):
        sl = slice(SPC * c, SPC * (c + 1))
        in_maps.append({
            "xt16": xt16[sl], "xt8": xt8[sl],
            "gw": gw_dev[sl].astype(f16), "epsd": eps_dev[sl],
            "id128": id128, "wpack": wpack,
        })
    return in_maps


def kernel(x, task_ids, eps, gate_w, fc1_w, fc1_b, fc2_w, fc2_b, _trace=False):
    nc = _build()
    in_maps = _prep_inputs(x, task_ids, eps, gate_w, fc1_w, fc1_b, fc2_w, fc2_b)
    res = run_bass_kernel_spmd(nc, in_maps, list(range(NCORES)), trace=_trace)
    yt = np.concatenate([res.results[c]["y"] for c in range(NCORES)], axis=0)
    # [B, 128, C_K, N] -> [B, N, C]
    out = np.ascontiguousarray(
        yt.astype(f32).transpose(0, 3, 2, 1)).reshape(B, N, C)
    kernel.last_results = res
    return out
